# revision 15
# baseline (speedup 1.0000x reference)
"""Llama4-style MoE (top-1 routing, 8 experts + shared SwiGLU) on 8 trn2 cores.

Sharding (expert-parallel + shared-expert tensor-parallel over F):
  Core c holds expert c's weights and an F/8 chunk of the shared expert,
  all pre-cast to bf16 on host. On device, each core:
    1. computes bf16 router logits for ALL tokens (replicated, cheap),
    2. top-1 argmax + fp32 sigmoid score per token,
    3. compacts the token-ids routed to ITS expert into <=CAP slots
       (mask -> cumsum-rank -> one-hot Z -> meta via one PE matmul),
       gathers those token rows (indirect DMA), scales by score, and runs
       the expert SwiGLU on the compacted [CAP, *] block in bf16,
    4. computes its F-chunk partial of the shared SwiGLU for all tokens.
  Outputs: partialT [H, T] bf16, routedT [H, CAP] bf16, meta [4, CAP] f32
  (slot -> in-tile row, validity, score, tile-id). Host sums partials,
  scatter-adds routed rows, and — since the device router runs in bf16 —
  recomputes exact fp32 routing on host and patches the few tokens whose
  argmax flipped (adds the correct expert's contribution in numpy fp32).

  PE stream order: router -> argmax transposes -> shared G/U (1st half)
  -> compaction + meta matmuls -> shared G/U (2nd half) -> gather
  transposes -> shared down -> routed G/U -> routed down. The routing
  chain (DVE/scalar/DMA) hides under shared-expert GEMMs, the tensor
  engine never idles >3us (keeps the HAM clock-gate warm), and PSUM
  drains alternate scalar/vector so neither becomes the pacer.
"""

from contextlib import ExitStack

import numpy as np
import ml_dtypes

import concourse.bass as bass
import concourse.mybir as mybir
import concourse.tile as tile
from concourse import bacc
from concourse.bass import IndirectOffsetOnAxis
from concourse.bass_utils import run_bass_kernel_spmd

BF16 = ml_dtypes.bfloat16

P = 128
T = 2048          # tokens
H = 1024          # hidden
F = 2048          # expert intermediate
E = 8             # experts == cores
FS = F // E       # shared-expert F chunk per core (256)
CAP = 288         # per-expert token capacity (max actual count is 277)
TT = T // P       # token tiles (16)
HT = H // P       # hidden tiles (8)
FT = F // P       # expert F tiles (16)
TCH = 512         # t-chunk (PSUM bank limit)
NTC = T // TCH    # 4
BIG = 1.0e6
GCH = 4           # ff per routed-G/U weight DMA chunk
DCH = 2           # hh per routed-down weight DMA chunk
BIGN = 1.0e6

f32 = mybir.dt.float32
bf16 = mybir.dt.bfloat16
i32 = mybir.dt.int32
AF = mybir.ActivationFunctionType
OP = mybir.AluOpType

N_CORES = 8
SLOT_ROWS = (P, P, CAP - 2 * P)   # gather tile sizes (128, 128, 32)


def _build_program():
    nc = bacc.Bacc(
        "TRN2",
        target_bir_lowering=False,
        debug=False,
        num_devices=N_CORES,
        enable_asserts=False,
    )

    # ---- I/O ----
    xh_d = nc.dram_tensor("xhT", [H, T], bf16, kind="ExternalInput")
    xb_d = nc.dram_tensor("xb", [T, H], bf16, kind="ExternalInput")
    gwb_d = nc.dram_tensor("gwb", [P, HT * E], bf16, kind="ExternalInput")
    sgb_d = nc.dram_tensor("sgb", [P, HT * FS], bf16, kind="ExternalInput")
    sub_d = nc.dram_tensor("sub", [P, HT * FS], bf16, kind="ExternalInput")
    sdb_d = nc.dram_tensor("sdb", [P, 2 * H], bf16, kind="ExternalInput")
    wg_d = nc.dram_tensor("wgp", [FT * P, H], bf16, kind="ExternalInput")
    wu_d = nc.dram_tensor("wup", [FT * P, H], bf16, kind="ExternalInput")
    wd_d = nc.dram_tensor("wdp", [HT * P, F], bf16, kind="ExternalInput")
    eid_d = nc.dram_tensor("eid", [P, 1], f32, kind="ExternalInput")
    idcb_d = nc.dram_tensor("idcolb", [P, 1], bf16, kind="ExternalInput")
    iob_d = nc.dram_tensor("iotaB", [P, CAP], f32, kind="ExternalInput")
    lsl_d = nc.dram_tensor("lsl", [TT, TT], f32, kind="ExternalInput")
    idb_d = nc.dram_tensor("identB", [P, P], bf16, kind="ExternalInput")
    idf_d = nc.dram_tensor("identF", [P, P], f32, kind="ExternalInput")

    pt_d = nc.dram_tensor("partialT", [H, T], bf16, kind="ExternalOutput")
    rt_d = nc.dram_tensor("routedT", [H, CAP], bf16, kind="ExternalOutput")
    mt_d = nc.dram_tensor("meta", [4, CAP], f32, kind="ExternalOutput")

    with tile.TileContext(nc) as tc, ExitStack() as ctx:
        pp = ctx.enter_context(tc.tile_pool(name="persist", bufs=1))
        wgp = ctx.enter_context(tc.tile_pool(name="wg", bufs=4))
        wup = ctx.enter_context(tc.tile_pool(name="wu", bufs=4))
        wdp = ctx.enter_context(tc.tile_pool(name="wd", bufs=4))
        xep = ctx.enter_context(tc.tile_pool(name="xe", bufs=2))
        xsp = ctx.enter_context(tc.tile_pool(name="xs", bufs=2))
        zp = ctx.enter_context(tc.tile_pool(name="z", bufs=2))
        gap = ctx.enter_context(tc.tile_pool(name="ga", bufs=2))
        smp = ctx.enter_context(tc.tile_pool(name="sm", bufs=4))
        ptp = ctx.enter_context(tc.tile_pool(name="pt", bufs=3))
        ps_s = ctx.enter_context(tc.tile_pool(name="ps_s", bufs=2, space="PSUM"))
        ps_a = ctx.enter_context(tc.tile_pool(name="ps_a", bufs=3, space="PSUM"))
        ps_g = ctx.enter_context(tc.tile_pool(name="ps_g", bufs=3, space="PSUM"))

        # ---- persistent SBUF ----
        # xh split per t-chunk so the router can start on chunk 0 arrival
        xh_c = [pp.tile([P, HT * TCH], bf16, name=f"xh{i}") for i in range(NTC)]
        gw_sb = pp.tile([P, HT * E], bf16)
        sg_sb = pp.tile([P, HT * FS], bf16)
        su_sb = pp.tile([P, HT * FS], bf16)
        sd_sb = pp.tile([P, 2 * H], bf16)
        idb_sb = pp.tile([P, P], bf16)
        idf_sb = pp.tile([P, P], f32)
        iob_sb = pp.tile([P, CAP], f32)
        idcb_sb = pp.tile([P, 1], bf16)
        eid_sb = pp.tile([P, 1], f32)
        lsl_sb = pp.tile([TT, TT], f32)
        lgt_sb = pp.tile([E, T], bf16)           # logitsT (bf16 for transposes)
        m16_sb = pp.tile([P, TT], f32)           # per-tile expert masks
        sc16_sb = pp.tile([P, TT], f32)          # per-tile scores (fp32)
        z16_sb = pp.tile([TT, P], f32)           # zeros for scan
        mt16_sb = pp.tile([TT, P], f32)
        cum_sb = pp.tile([TT, P], f32)
        rk_sb = pp.tile([TT, P], f32)
        rc_sb = pp.tile([P, TT], f32)            # masked rank, token-tile cols
        mew_sb = pp.tile([4, CAP], f32)          # meta (row, valid, score, tile)
        idx_sb = pp.tile([P, 3], i32)            # gather indices per slot tile
        scc_sb = pp.tile([P, 3], f32)            # per-slot scores
        xst_sb = pp.tile([P, HT * CAP], bf16)    # compacted tokens, transposed
        ash_sb = pp.tile([P, 2 * T], bf16)       # shared act
        ar_sb = pp.tile([P, FT * CAP], bf16)     # routed act
        rta_sb = pp.tile([P, HT * CAP], bf16)    # routed down staging

        # ---- input loads (sync = HWDGE ring 1), in consumption order ----
        nc.sync.dma_start(out=gw_sb[:], in_=gwb_d.ap()[:])
        nc.sync.dma_start(out=idb_sb[:], in_=idb_d.ap()[:])
        xh_dr = xh_d.ap().rearrange("(a p) t -> p a t", p=P)
        nc.sync.dma_start(
            out=xh_c[0][:].rearrange("p (a t) -> p a t", a=HT),
            in_=xh_dr[:, :, 0:TCH],
        )
        nc.sync.dma_start(out=eid_sb[:], in_=eid_d.ap()[:])
        nc.sync.dma_start(out=idf_sb[:], in_=idf_d.ap()[:])
        nc.sync.dma_start(out=iob_sb[:], in_=iob_d.ap()[:])
        nc.sync.dma_start(out=idcb_sb[:], in_=idcb_d.ap()[:])
        nc.sync.dma_start(out=lsl_sb[:], in_=lsl_d.ap()[:])
        nc.sync.dma_start(
            out=xh_c[1][:].rearrange("p (a t) -> p a t", a=HT),
            in_=xh_dr[:, :, TCH:2 * TCH],
        )
        nc.sync.dma_start(out=sg_sb[:], in_=sgb_d.ap()[:])
        nc.sync.dma_start(
            out=xh_c[2][:].rearrange("p (a t) -> p a t", a=HT),
            in_=xh_dr[:, :, 2 * TCH:3 * TCH],
        )
        nc.sync.dma_start(out=su_sb[:], in_=sub_d.ap()[:])
        nc.sync.dma_start(
            out=xh_c[3][:].rearrange("p (a t) -> p a t", a=HT),
            in_=xh_dr[:, :, 3 * TCH:4 * TCH],
        )
        nc.sync.dma_start(out=sd_sb[:], in_=sdb_d.ap()[:])
        nc.gpsimd.memset(z16_sb[:], 0.0)

        # routed expert weights: all chunks resident (bufs cover them), down
        # weights first so nothing arrives late behind the G/U stream
        wd_t = []
        for cs in range(HT // DCH):
            wt = wdp.tile([P, DCH * F], bf16, tag="wd")
            nc.sync.dma_start(
                out=wt[:].rearrange("p (a c) -> p a c", a=DCH),
                in_=wd_d.ap()[cs * DCH * P:(cs + 1) * DCH * P, :].rearrange(
                    "(a p) c -> p a c", p=P),
            )
            wd_t.append(wt)
        wg_t = []
        for cs in range(FT // GCH):
            wt = wgp.tile([P, GCH * H], bf16, tag="wg")
            nc.sync.dma_start(
                out=wt[:].rearrange("p (a c) -> p a c", a=GCH),
                in_=wg_d.ap()[cs * GCH * P:(cs + 1) * GCH * P, :].rearrange(
                    "(a p) c -> p a c", p=P),
            )
            wu_t = wup.tile([P, GCH * H], bf16, tag="wu")
            nc.sync.dma_start(
                out=wu_t[:].rearrange("p (a c) -> p a c", a=GCH),
                in_=wu_d.ap()[cs * GCH * P:(cs + 1) * GCH * P, :].rearrange(
                    "(a p) c -> p a c", p=P),
            )
            wg_t.append((wt, wu_t))

        # ---- router: logitsT[e, t] = sum_h gwT[h, e] * xhT[h, t] (bf16) ----
        for tc_i in range(NTC):
            ps = ps_s.tile([E, TCH], f32, space="PSUM", tag="pss")
            for hh in range(HT):
                nc.tensor.matmul(
                    out=ps[:],
                    lhsT=gw_sb[:, hh * E:(hh + 1) * E],
                    rhs=xh_c[tc_i][:, hh * TCH:(hh + 1) * TCH],
                    start=(hh == 0),
                    stop=(hh == HT - 1),
                )
            nc.vector.tensor_copy(
                out=lgt_sb[:, tc_i * TCH:(tc_i + 1) * TCH], in_=ps[:]
            )

        # ---- per-token argmax / score / mask ----
        for tt in range(TT):
            trp = ps_s.tile([P, E], bf16, space="PSUM", tag="pss")
            nc.tensor.transpose(
                out=trp[:],
                in_=lgt_sb[:, tt * P:(tt + 1) * P],
                identity=idb_sb[0:E, 0:E],
            )
            lg = smp.tile([P, E], f32)
            nc.vector.tensor_copy(out=lg[:], in_=trp[:])
            mx = smp.tile([P, E], f32)
            mi = smp.tile([P, E], mybir.dt.uint32)
            nc.vector.max(out=mx[:], in_=lg[:])
            nc.vector.max_index(out=mi[:], in_max=mx[:], in_values=lg[:])
            nc.scalar.activation(
                out=sc16_sb[:, tt:tt + 1], in_=mx[:, 0:1], func=AF.Sigmoid
            )
            tidf = smp.tile([P, 1], f32)
            nc.vector.tensor_copy(out=tidf[:], in_=mi[:, 0:1])
            nc.vector.tensor_tensor(
                out=m16_sb[:, tt:tt + 1], in0=tidf[:], in1=eid_sb[:],
                op=OP.is_equal,
            )

        def shared_gu(ff):
            for tc_i in range(NTC):
                psg = ps_a.tile([P, TCH], f32, space="PSUM", tag="psa")
                for hh in range(HT):
                    nc.tensor.matmul(
                        out=psg[:],
                        lhsT=sg_sb[:, hh * FS + ff * P: hh * FS + (ff + 1) * P],
                        rhs=xh_c[tc_i][:, hh * TCH:(hh + 1) * TCH],
                        start=(hh == 0),
                        stop=(hh == HT - 1),
                    )
                psu = ps_a.tile([P, TCH], f32, space="PSUM", tag="psa")
                for hh in range(HT):
                    nc.tensor.matmul(
                        out=psu[:],
                        lhsT=su_sb[:, hh * FS + ff * P: hh * FS + (ff + 1) * P],
                        rhs=xh_c[tc_i][:, hh * TCH:(hh + 1) * TCH],
                        start=(hh == 0),
                        stop=(hh == HT - 1),
                    )
                ga = gap.tile([P, TCH], f32, tag="ga")
                nc.scalar.activation(out=ga[:], in_=psg[:], func=AF.Silu)
                nc.vector.tensor_tensor(
                    out=ash_sb[:, ff * T + tc_i * TCH: ff * T + (tc_i + 1) * TCH],
                    in0=ga[:], in1=psu[:], op=OP.mult,
                )

        # ---- shared expert G/U, first half (hides the DVE routing chain) ----
        shared_gu(0)

        # ---- compaction: global rank of each of my tokens ----
        mt_ps = ps_s.tile([TT, P], f32, space="PSUM", tag="pss")
        nc.tensor.transpose(out=mt_ps[:], in_=m16_sb[:], identity=idf_sb[:])
        nc.vector.tensor_copy(out=mt16_sb[:], in_=mt_ps[:])
        nc.vector.tensor_tensor_scan(
            out=cum_sb[:], data0=mt16_sb[:], data1=z16_sb[:],
            initial=0.0, op0=OP.add, op1=OP.add,
        )
        off_ps = ps_s.tile([TT, 1], f32, space="PSUM", tag="pss")
        nc.tensor.matmul(
            out=off_ps[:], lhsT=lsl_sb[:], rhs=cum_sb[:, P - 1:P],
            start=True, stop=True,
        )
        off_sb = smp.tile([TT, 1], f32)
        nc.vector.tensor_copy(out=off_sb[:], in_=off_ps[:])
        # rank0_masked = cum + off - 1 + BIG*(1 - m)
        t1 = smp.tile([TT, P], f32)
        nc.vector.tensor_scalar(
            out=t1[:], in0=cum_sb[:], scalar1=off_sb[:], scalar2=BIG - 1.0,
            op0=OP.add, op1=OP.add,
        )
        t2 = smp.tile([TT, P], f32)
        nc.vector.tensor_scalar_mul(t2[:], mt16_sb[:], BIG)
        nc.vector.tensor_tensor(
            out=rk_sb[:], in0=t1[:], in1=t2[:], op=OP.subtract
        )
        rk_ps = ps_s.tile([P, TT], f32, space="PSUM", tag="pss")
        nc.tensor.transpose(
            out=rk_ps[:], in_=rk_sb[:], identity=idf_sb[0:TT, 0:TT]
        )
        nc.vector.tensor_copy(out=rc_sb[:], in_=rk_ps[:])

        # ---- meta matmul: in-tile row / valid / score / tile per slot ----
        me_ps = ps_s.tile([4, CAP], f32, space="PSUM", tag="pss")
        for tt in range(TT):
            z = zp.tile([P, CAP], bf16, tag="z")
            nc.vector.tensor_tensor(
                out=z[:],
                in0=rc_sb[:, tt:tt + 1].to_broadcast([P, CAP]),
                in1=iob_sb[:],
                op=OP.is_equal,
            )
            l4 = smp.tile([P, 4], bf16)
            nc.vector.tensor_copy(out=l4[:, 0:1], in_=idcb_sb[:])
            nc.gpsimd.memset(l4[:, 1:2], 1.0)
            nc.vector.tensor_copy(out=l4[:, 2:3], in_=sc16_sb[:, tt:tt + 1])
            nc.gpsimd.memset(l4[:, 3:4], float(tt))
            nc.tensor.matmul(
                out=me_ps[:], lhsT=l4[:], rhs=z[:],
                start=(tt == 0), stop=(tt == TT - 1),
            )
        nc.vector.tensor_copy(out=mew_sb[:], in_=me_ps[:])
        nc.scalar.dma_start(out=mt_d.ap()[:], in_=mew_sb[:])
        for k in range(3):
            kn = SLOT_ROWS[k]
            pc_ps = ps_s.tile([P, 4], f32, space="PSUM", tag="pss")
            nc.tensor.transpose(
                out=pc_ps[0:kn, :],
                in_=mew_sb[:, k * P:k * P + kn],
                identity=idf_sb[0:4, 0:4],
            )
            pc = smp.tile([P, 4], f32)
            nc.vector.tensor_copy(out=pc[0:kn, :], in_=pc_ps[0:kn, :])
            idf_t = smp.tile([P, 1], f32)
            nc.vector.tensor_scalar(
                out=idf_t[0:kn, :], in0=pc[0:kn, 3:4], scalar1=float(P),
                scalar2=None, op0=OP.mult,
            )
            nc.vector.tensor_tensor(
                out=idf_t[0:kn, :], in0=idf_t[0:kn, :], in1=pc[0:kn, 0:1],
                op=OP.add,
            )
            nc.vector.tensor_copy(out=idx_sb[0:kn, k:k + 1], in_=idf_t[0:kn, :])
            nc.vector.tensor_copy(out=scc_sb[0:kn, k:k + 1], in_=pc[0:kn, 2:3])

        # ---- gather + scale the expert's tokens (DMA/DVE, overlaps PE) ----
        xs_t = []
        for k in range(3):
            kn = SLOT_ROWS[k]
            xe = xep.tile([P, H], bf16, tag="xe")
            nc.gpsimd.indirect_dma_start(
                out=xe[0:kn, :],
                out_offset=None,
                in_=xb_d.ap()[:],
                in_offset=IndirectOffsetOnAxis(ap=idx_sb[0:kn, k:k + 1], axis=0),
            )
            xs = xsp.tile([P, H], bf16, tag="xs")
            nc.vector.tensor_scalar_mul(xs[0:kn, :], xe[0:kn, :],
                                        scc_sb[0:kn, k:k + 1])
            xs_t.append(xs)

        # ---- shared expert G/U, second half ----
        shared_gu(1)

        # ---- transpose compacted tokens to [h, slot] layout ----
        for k in range(3):
            kn = SLOT_ROWS[k]
            xs = xs_t[k]
            for hh in range(HT):
                tp = ps_s.tile([P, P], bf16, space="PSUM", tag="pss")
                nc.tensor.transpose(
                    out=tp[:, 0:kn], in_=xs[0:kn, hh * P:(hh + 1) * P],
                    identity=idb_sb[0:kn, 0:kn],
                )
                if hh % 2 == 0:
                    nc.scalar.activation(
                        out=xst_sb[:, hh * CAP + k * P: hh * CAP + k * P + kn],
                        in_=tp[:, 0:kn], func=AF.Copy,
                    )
                else:
                    nc.vector.tensor_copy(
                        out=xst_sb[:, hh * CAP + k * P: hh * CAP + k * P + kn],
                        in_=tp[:, 0:kn],
                    )

        # ---- shared down -> partialT (writes overlap routed phase) ----
        for hh in range(HT):
            ptst = ptp.tile([P, T], bf16, tag="pt")
            for tc_i in range(NTC):
                ps2 = ps_a.tile([P, TCH], f32, space="PSUM", tag="psa")
                for ffp in range(2):
                    nc.tensor.matmul(
                        out=ps2[:],
                        lhsT=sd_sb[:, ffp * H + hh * P: ffp * H + (hh + 1) * P],
                        rhs=ash_sb[:, ffp * T + tc_i * TCH: ffp * T + (tc_i + 1) * TCH],
                        start=(ffp == 0),
                        stop=(ffp == 1),
                    )
                if tc_i % 2 == 0:
                    nc.scalar.activation(
                        out=ptst[:, tc_i * TCH:(tc_i + 1) * TCH], in_=ps2[:],
                        func=AF.Copy,
                    )
                else:
                    nc.vector.tensor_copy(
                        out=ptst[:, tc_i * TCH:(tc_i + 1) * TCH], in_=ps2[:],
                    )
            nc.scalar.dma_start(
                out=pt_d.ap()[hh * P:(hh + 1) * P, :], in_=ptst[:]
            )

        # ---- routed expert: G/U + act on compacted tokens (bf16) ----
        for ff in range(FT):
            wt, wu_t = wg_t[ff // GCH]
            fo = (ff % GCH) * H
            psg = ps_g.tile([P, CAP], f32, space="PSUM", tag="psg")
            for hh in range(HT):
                nc.tensor.matmul(
                    out=psg[:],
                    lhsT=wt[:, fo + hh * P: fo + (hh + 1) * P],
                    rhs=xst_sb[:, hh * CAP:(hh + 1) * CAP],
                    start=(hh == 0),
                    stop=(hh == HT - 1),
                )
            psu = ps_g.tile([P, CAP], f32, space="PSUM", tag="psg")
            for hh in range(HT):
                nc.tensor.matmul(
                    out=psu[:],
                    lhsT=wu_t[:, fo + hh * P: fo + (hh + 1) * P],
                    rhs=xst_sb[:, hh * CAP:(hh + 1) * CAP],
                    start=(hh == 0),
                    stop=(hh == HT - 1),
                )
            ga = gap.tile([P, CAP], f32, tag="ga")
            nc.scalar.activation(out=ga[:], in_=psg[:], func=AF.Silu)
            nc.vector.tensor_tensor(
                out=ar_sb[:, ff * CAP:(ff + 1) * CAP],
                in0=ga[:], in1=psu[:], op=OP.mult,
            )

        # ---- routed down -> routedT (per-hh writes shrink the tail) ----
        rt_dr = rt_d.ap().rearrange("(a p) c -> p a c", p=P)
        for hh in range(HT):
            wt = wd_t[hh // DCH]
            ho = (hh % DCH) * F
            ps3 = ps_g.tile([P, CAP], f32, space="PSUM", tag="psg")
            for ff in range(FT):
                nc.tensor.matmul(
                    out=ps3[:],
                    lhsT=wt[:, ho + ff * P: ho + (ff + 1) * P],
                    rhs=ar_sb[:, ff * CAP:(ff + 1) * CAP],
                    start=(ff == 0),
                    stop=(ff == FT - 1),
                )
            if hh % 2 == 0:
                nc.scalar.activation(
                    out=rta_sb[:, hh * CAP:(hh + 1) * CAP], in_=ps3[:],
                    func=AF.Copy,
                )
            else:
                nc.vector.tensor_copy(
                    out=rta_sb[:, hh * CAP:(hh + 1) * CAP], in_=ps3[:]
                )
            nc.scalar.dma_start(
                out=rt_dr[:, hh:hh + 1, :],
                in_=rta_sb[:, hh * CAP:(hh + 1) * CAP].rearrange(
                    "p (a c) -> p a c", a=1),
            )

    nc.compile()
    return nc


_PROGRAM = None


def _get_program():
    global _PROGRAM
    if _PROGRAM is None:
        _PROGRAM = _build_program()
    return _PROGRAM


def blk(w_t):
    # w_t: [K, M] (contraction-major). Returns [M_tiles*P, K]: rows
    # [m*P:(m+1)*P] hold the lhsT chunk for output tile m, k-tile-major.
    kt_, mt_ = w_t.shape[0] // P, w_t.shape[1] // P
    return np.ascontiguousarray(
        w_t.reshape(kt_, P, mt_, P).transpose(2, 1, 0, 3).reshape(
            mt_ * P, kt_ * P)
    ).astype(BF16)


def _prep_inputs(x32, gw32, sg, su, sd, rg, ru, rd):
    xb = x32.astype(BF16)
    xhT = np.ascontiguousarray(xb.T)                       # [H, T] bf16
    gwT = np.ascontiguousarray(gw32.T)                     # [H, E]
    gwb = np.ascontiguousarray(
        gwT.reshape(HT, P, E).transpose(1, 0, 2).reshape(P, HT * E)
    ).astype(BF16)
    iotaB = np.broadcast_to(
        np.arange(CAP, dtype=np.float32)[None, :], (P, CAP)).copy()
    idcolb = np.arange(P, dtype=np.float32)[:, None].astype(BF16)
    lsl = np.triu(np.ones((TT, TT), dtype=np.float32), k=1)
    identB = np.eye(P, dtype=np.float32).astype(BF16)
    identF = np.eye(P, dtype=np.float32)

    in_maps = []
    for c in range(N_CORES):
        fsl = slice(c * FS, (c + 1) * FS)
        sgT = sg[fsl, :].T                                 # [H, FS]
        suT = su[fsl, :].T
        sdT = sd[:, fsl].T                                 # [FS, H]
        sgb = np.ascontiguousarray(
            sgT.reshape(HT, P, FS).transpose(1, 0, 2).reshape(P, HT * FS)
        ).astype(BF16)
        sub = np.ascontiguousarray(
            suT.reshape(HT, P, FS).transpose(1, 0, 2).reshape(P, HT * FS)
        ).astype(BF16)
        sdb = np.ascontiguousarray(
            sdT.reshape(2, P, H).transpose(1, 0, 2).reshape(P, 2 * H)
        ).astype(BF16)
        in_maps.append({
            "xhT": xhT,
            "xb": xb,
            "gwb": gwb,
            "sgb": sgb,
            "sub": sub,
            "sdb": sdb,
            "wgp": blk(rg[c].T),                           # rg[c].T: [H, F]
            "wup": blk(ru[c].T),
            "wdp": blk(rd[c].T),                           # rd[c].T: [F, H]
            "eid": np.full((P, 1), float(c), dtype=np.float32),
            "idcolb": idcolb,
            "iotaB": iotaB,
            "lsl": lsl,
            "identB": identB,
            "identF": identF,
        })
    return in_maps


def _silu(v):
    return v / (1.0 + np.exp(-v))


def kernel(hidden_states, gate_w, shared_gate, shared_up, shared_down,
           r_gate, r_up, r_down, _trace=False):
    x32 = np.ascontiguousarray(
        np.asarray(hidden_states, dtype=np.float32).reshape(T, H))
    gw32 = np.asarray(gate_w, dtype=np.float32)
    sg = np.asarray(shared_gate, dtype=np.float32)
    su = np.asarray(shared_up, dtype=np.float32)
    sd = np.asarray(shared_down, dtype=np.float32)
    rg = np.asarray(r_gate, dtype=np.float32)
    ru = np.asarray(r_up, dtype=np.float32)
    rd = np.asarray(r_down, dtype=np.float32)

    nc = _get_program()
    in_maps = _prep_inputs(x32, gw32, sg, su, sd, rg, ru, rd)
    res = run_bass_kernel_spmd(nc, in_maps, list(range(N_CORES)), trace=_trace)

    out_t = np.zeros((H, T), dtype=np.float32)
    for c in range(N_CORES):
        out_t += res.results[c]["partialT"].astype(np.float32)
    out = np.ascontiguousarray(out_t.T)                    # [T, H]

    # exact fp32 routing on host; patch tokens whose bf16 argmax flipped
    logits = x32 @ gw32.T
    ref_top = logits.argmax(-1)
    ref_score = 1.0 / (1.0 + np.exp(-logits.max(-1)))

    handled = np.zeros(T, dtype=bool)
    for c in range(N_CORES):
        meta = np.asarray(res.results[c]["meta"], dtype=np.float32)
        routed = res.results[c]["routedT"].astype(np.float32).T  # [CAP, H]
        valid = meta[1] > 0.5
        tok = (meta[0] + P * meta[3]).astype(np.int64)
        ok = valid & (ref_top[np.clip(tok, 0, T - 1)] == c)
        out[tok[ok]] += routed[ok]
        handled[tok[ok]] = True

    for t in np.nonzero(~handled)[0]:
        e = int(ref_top[t])
        xs = x32[t] * ref_score[t]
        a = _silu(rg[e] @ xs) * (ru[e] @ xs)
        out[t] += rd[e] @ a

    out = out.reshape(1, T, H)
    if _trace:
        return out, res
    return out
